# revision 10
# baseline (speedup 1.0000x reference)
"""Trainium2 Bass kernel for gated pair-bias attention (AlphaFold-style).

Reference computation per (b=1, n) row:
  q,k,v = proj(input_*) reshaped to [H=8, S=256, C=32]; q /= sqrt(32)
  a = softmax(q@k^T + (mask-1)*1e9 + bias)      # [H, Q, K]
  o = (a@v) * sigmoid(input_q@wg + bg)          # gated
  out = o @ wo + bo                             # [S, 128]

Sharding: dim 1 (N=256 rows) split across 8 cores, 32 rows/core.

Engine-balanced design (v3). Baseline was Act(exp)+DVE(bias-mult) bound:
  - direct qT/kT (no block-diag q): per-head logits matmuls (c=32, m=256)
  - exp(l)*exp(b) product form for 3 of 4 logit tiles (Act engine),
    4th tile via the i16 fast-exp pun on DVE: a single AFFINE_THEN_ADD
    custom op computes i16 = l*(128/ln2) + (b*(128/ln2) + 16256 - 7.5),
    whose int16 bits reinterpreted as bf16 equal exp(l+b) to ~2%.
  - gate computed transposed (gT = tanh-sigmoid of wg^T x), fixup on DVE
  - AV natural with Z ride-along column (v_aug | 1), PSUM per q-chunk
  - normalize (1/Z) on Pool, PE transpose, gate-mult on DVE (2x bf16)
  - final matmul from og^T, bias-add + evac on Pool, DMA out
Work is spread so PE/Act/DVE/Pool all run ~equally busy.
"""

import math
import sys

sys.path.insert(0, "/opt/trn_rl_repo")

import numpy as np
import ml_dtypes

BF16 = ml_dtypes.bfloat16

B, N, S, CQ = 1, 256, 256, 128
H, C = 8, 32
NCORES = 8
NPER = N // NCORES  # 32 rows per core

FE_A = 128.0 / math.log(2.0)   # fast-exp scale
FE_B = 127.0 * 128.0 - 7.5     # fast-exp bias (incl. sawtooth centering)
USE_FAST_EXP = True


def _build_bass():
    import concourse.bass as bass
    import concourse.bacc as bacc
    import concourse.tile as tile
    from concourse import mybir
    from concourse.masks import make_identity
    from concourse.dve_ops import AFFINE_THEN_ADD

    dt = mybir.dt
    AF = mybir.ActivationFunctionType
    ALU = mybir.AluOpType

    nc = bacc.Bacc()

    x_all = nc.declare_dram_parameter("x_all", [NPER, 3, CQ, S], dt.bfloat16, isOutput=False)
    wq_d = nc.declare_dram_parameter("wq", [CQ, 2, CQ], dt.bfloat16, isOutput=False)
    wk_d = nc.declare_dram_parameter("wk", [CQ, 2, CQ], dt.bfloat16, isOutput=False)
    wv_d = nc.declare_dram_parameter("wv", [CQ, 2 * CQ], dt.bfloat16, isOutput=False)
    wg_d = nc.declare_dram_parameter("wg", [CQ, 2, CQ], dt.bfloat16, isOutput=False)
    wo_d = nc.declare_dram_parameter("wo", [CQ, 2, CQ], dt.bfloat16, isOutput=False)
    eb_d = nc.declare_dram_parameter("eb2", [CQ, 2, 4 * S], dt.bfloat16, isOutput=False)
    bA_d = nc.declare_dram_parameter("bA", [CQ, 2, 4 * S], dt.float32, isOutput=False)
    ebB_d = nc.declare_dram_parameter("ebB", [CQ, 2, 4 * S], dt.bfloat16, isOutput=False)
    bgt_d = nc.declare_dram_parameter("bgt", [CQ, 2], dt.float32, isOutput=False)
    bo_d = nc.declare_dram_parameter("bo", [1, CQ], dt.float32, isOutput=False)
    emf_d = nc.declare_dram_parameter("emf", [CQ, 2 * NPER], dt.float32, isOutput=False)
    out_d = nc.declare_dram_parameter("out", [NPER, S, CQ], dt.float32, isOutput=True)

    with tile.TileContext(nc) as tc:
        with (
            tc.tile_pool(name="const", bufs=1) as const,
            tc.tile_pool(name="xp", bufs=3) as xp,
            tc.tile_pool(name="qkp", bufs=2) as qkp,
            tc.tile_pool(name="vap", bufs=2) as vap,
            tc.tile_pool(name="gp", bufs=2) as gpool,
            tc.tile_pool(name="esb", bufs=6) as esb,
            tc.tile_pool(name="ogp", bufs=3) as ogp,
            tc.tile_pool(name="zp", bufs=3) as zp,
            tc.tile_pool(name="outp", bufs=2) as outp,
            tc.tile_pool(name="pse", bufs=2, space="PSUM") as pse,
            tc.tile_pool(name="pso", bufs=2, space="PSUM") as pso,
            tc.tile_pool(name="psx", bufs=1, space="PSUM") as psx,
        ):
            # ---------- once-per-core setup ----------
            wq_t = const.tile([CQ, 2, CQ], dt.bfloat16, tag="wq")
            wk_t = const.tile([CQ, 2, CQ], dt.bfloat16, tag="wk")
            wv_t = const.tile([CQ, 2 * CQ], dt.bfloat16, tag="wv")
            wg_t = const.tile([CQ, 2, CQ], dt.bfloat16, tag="wg")
            wo_t = const.tile([CQ, 2, CQ], dt.bfloat16, tag="wo")
            eb_t = const.tile([CQ, 2, 4 * S], dt.bfloat16, tag="eb")
            bA_t = const.tile([CQ, 2, 4 * S], dt.float32, tag="bA")
            ebB_t = const.tile([CQ, 2, 4 * S], dt.bfloat16, tag="ebB")
            bgt_t = const.tile([CQ, 2], dt.float32, tag="bgt")
            bo_t = const.tile([CQ, CQ], dt.float32, tag="bo")
            emf = const.tile([CQ, 2, NPER], dt.float32, tag="emf")
            emb = const.tile([CQ, 2, NPER], dt.bfloat16, tag="emb")
            ident = const.tile([CQ, CQ], dt.bfloat16, tag="ident")

            nc.sync.dma_start(out=wq_t, in_=wq_d[:, :])
            nc.sync.dma_start(out=wk_t, in_=wk_d[:, :])
            nc.sync.dma_start(out=wv_t, in_=wv_d[:, :])
            nc.sync.dma_start(out=wg_t, in_=wg_d[:, :])
            nc.sync.dma_start(out=wo_t, in_=wo_d[:, :])
            nc.sync.dma_start(out=eb_t, in_=eb_d[:, :])
            nc.sync.dma_start(out=bA_t, in_=bA_d[:, :])
            nc.sync.dma_start(out=ebB_t, in_=ebB_d[:, :])
            nc.sync.dma_start(out=bgt_t, in_=bgt_d[:, :])
            nc.sync.dma_start(out=emf, in_=emf_d.rearrange("p (k n) -> p k n", k=2))
            bo_ap0 = bo_d[:, :]
            bo_bc_ap = bass.AP(tensor=bo_ap0.tensor, offset=bo_ap0.offset,
                               ap=[[0, CQ], [1, CQ]])
            nc.sync.dma_start(out=bo_t, in_=bo_bc_ap)
            make_identity(nc, ident)
            nc.vector.tensor_copy(out=emb, in_=emf)

            # ---------- per-row pipeline ----------
            for n in range(NPER):
                xt = xp.tile([CQ, 3, S], dt.bfloat16, tag="xt")
                nc.sync.dma_start(out=xt, in_=x_all[n].rearrange("t p s -> p t s"))

                # qT/kT [d, s] in one 2-bank PSUM tile: [:,0:2]=qT dc, [:,2:4]=kT dc
                pqk = pse.tile([CQ, 4, S], dt.float32, tag="pse")
                for dc in range(2):
                    nc.tensor.matmul(pqk[:, dc], wq_t[:, dc], xt[:, 0],
                                     start=True, stop=True)
                    nc.tensor.matmul(pqk[:, 2 + dc], wk_t[:, dc], xt[:, 1],
                                     start=True, stop=True)
                qkT = qkp.tile([CQ, 4, S], dt.bfloat16, tag="qkT")
                nc.scalar.activation(qkT.rearrange("p a s -> p (a s)"),
                                     pqk.rearrange("p a s -> p (a s)"), AF.Copy)
                # per-head shift: head-block g (partitions g*32..) -> partitions
                # 0-31 of qkT_s[:, g] so logits matmuls read base-0 operands
                qkT_s = qkp.tile([32, 4, 4, S], dt.bfloat16, tag="qkTs")
                for g in range(4):
                    nc.gpsimd.dma_start(out=qkT_s[:, g],
                                        in_=qkT[g * 32:(g + 1) * 32])

                # v natural [k, hd] per kc + gate transposed [hd, q] per dc
                pvg = pse.tile([CQ, 4, S], dt.float32, tag="pse")
                for kc in range(2):
                    nc.tensor.matmul(pvg[:, kc], xt[:, 2, kc * CQ:(kc + 1) * CQ],
                                     wv_t, start=True, stop=True)
                for dc in range(2):
                    nc.tensor.matmul(pvg[:, 2 + dc], wg_t[:, dc], xt[:, 0],
                                     start=True, stop=True)

                # v_aug [k, kc, h, 33]: v*em | em  (em==1 when mask is ones)
                va = vap.tile([CQ, 2, H, 33], dt.bfloat16, tag="va")
                for kc in range(2):
                    nc.scalar.activation(
                        va[:, kc, :, 0:32],
                        pvg[:, kc].rearrange("p (h x) -> p h x", x=32),
                        AF.Copy, scale=emf[:, kc, n:n + 1])
                nc.gpsimd.tensor_copy(
                    out=va[:, :, :, 32],
                    in_=emb[:, :, n:n + 1].broadcast_to((CQ, 2, H)))

                # gate: tanh((z+bg)/2) on Act; 0.5*t+0.5 fixup on DVE
                gt = gpool.tile([CQ, 2, S], dt.bfloat16, tag="gt")
                for dc in range(2):
                    nc.scalar.activation(gt[:, dc], pvg[:, 2 + dc], AF.Tanh,
                                         scale=0.5, bias=bgt_t[:, dc:dc + 1])
                g_sb = gpool.tile([CQ, 2, S], dt.bfloat16, tag="g")
                nc.gpsimd.tensor_scalar(out=g_sb, in0=gt, scalar1=0.5, scalar2=0.5,
                                        op0=ALU.mult, op1=ALU.add)

                # logits per (dc,kc): 4 heads, e^T[k, (h, q)]
                e_sb = {}
                for dc in range(2):
                    for kc in range(2):
                        pe_e = pse.tile([CQ, 4, S], dt.float32, tag="pse")
                        for hh in range(4):
                            nc.tensor.matmul(
                                pe_e[:, hh],
                                qkT_s[:, hh, 2 + dc, kc * CQ:(kc + 1) * CQ],
                                qkT_s[:, hh, dc, :],
                                start=True, stop=True)
                        et = esb.tile([CQ, 4, S], dt.bfloat16, tag="et")
                        flat_in = pe_e.rearrange("p h s -> p (h s)")
                        flat_out = et.rearrange("p h s -> p (h s)")
                        if dc == 1 and USE_FAST_EXP:
                            # fast-exp: i16 = l*A + (b*A + B); bits = bf16 e^(l+b)
                            nc.vector._custom_dve(
                                AFFINE_THEN_ADD,
                                out=flat_out.bitcast(dt.int16),
                                in0=flat_in, in1=bA_t[:, kc],
                                s0=FE_A, s1=FE_B)
                        elif dc == 1:
                            nc.scalar.activation(flat_out, flat_in, AF.Exp)
                            eng = nc.vector if kc == 0 else nc.gpsimd
                            eng.tensor_mul(flat_out, flat_out, ebB_t[:, kc])
                        else:
                            nc.scalar.activation(flat_out, flat_in, AF.Exp)
                            eng = nc.vector if kc == 0 else nc.gpsimd
                            eng.tensor_mul(
                                flat_out, flat_out, eb_t[:, kc])
                        e_sb[(dc, kc)] = et

                # AV with Z column: o[q, (h,33)] per q-chunk
                pos = []
                for qc in range(2):
                    po = pso.tile([CQ, H, 33], dt.float32, tag="pso")
                    for h in range(H):
                        dc, hh = h // 4, h % 4
                        for kc in range(2):
                            nc.tensor.matmul(
                                po[:, h],
                                e_sb[(dc, kc)][:, hh, qc * CQ:(qc + 1) * CQ],
                                va[:, kc, h],
                                start=(kc == 0), stop=(kc == 1))
                    pos.append(po)

                # normalize: rz = 1/Z (DVE), og1 = o * rz (Pool)
                rz = zp.tile([CQ, 2, H], dt.float32, tag="rz")
                og1 = ogp.tile([CQ, 2, H, 32], dt.bfloat16, tag="og1")
                for qc in range(2):
                    nc.vector.reciprocal(out=rz[:, qc], in_=pos[qc][:, :, 32])
                    nc.vector.tensor_mul(
                        og1[:, qc], pos[qc][:, :, 0:32],
                        rz[:, qc].unsqueeze(2).broadcast_to((CQ, H, 32)))

                # og^T via PE transpose, gate-mult on DVE (bf16 2x)
                ptp = psx.tile([CQ, 2, 2, CQ], dt.bfloat16, tag="ptp")
                og1f = og1.rearrange("p a h x -> p a (h x)")
                for dc in range(2):
                    for qc in range(2):
                        nc.tensor.transpose(
                            ptp[:, dc, qc],
                            og1f[:, qc, dc * CQ:(dc + 1) * CQ], ident)
                ogT = ogp.tile([CQ, 2, 2, CQ], dt.bfloat16, tag="ogT")
                for dc in range(2):
                    nc.vector.tensor_mul(ogT[:, dc], ptp[:, dc],
                                         g_sb[:, dc].rearrange("p (a c) -> p a c", c=CQ))

                # final: out[q, c] = og^T.T @ wo + bo
                pout = psx.tile([CQ, 2, CQ], dt.float32, tag="pout")
                for qc in range(2):
                    for dc in range(2):
                        nc.tensor.matmul(pout[:, qc], ogT[:, dc, qc], wo_t[:, dc],
                                         start=(dc == 0), stop=(dc == 1))
                out_sb = outp.tile([CQ, 2, CQ], dt.float32, tag="osb")
                nc.vector.tensor_add(
                    out_sb, pout, bo_t.unsqueeze(1).broadcast_to((CQ, 2, CQ)))
                nc.sync.dma_start(
                    out=out_d[n].rearrange("(qc p) c -> p qc c", p=CQ),
                    in_=out_sb)
    if not nc.is_finalized():
        nc.finalize()
    return nc


_NC_CACHE = None


def _get_nc():
    global _NC_CACHE
    if _NC_CACHE is None:
        _NC_CACHE = _build_bass()
    return _NC_CACHE


def kernel(input_q, input_k, input_v, mask, bias, wq, wk, wv, wg, bg, wo, bo):
    from concourse.bass_utils import run_bass_kernel_spmd

    nc = _get_nc()

    # ---- host-side prep (sharding + layout) ----
    wq_s = np.ascontiguousarray(
        (wq / math.sqrt(C)).reshape(CQ, 2, CQ)).astype(BF16)
    wk_s = np.ascontiguousarray(wk.reshape(CQ, 2, CQ)).astype(BF16)
    wv_s = wv.astype(BF16)
    wg_s = np.ascontiguousarray(wg.reshape(CQ, 2, CQ)).astype(BF16)
    # wo [hd, c] -> [p, dc, c] with hd = dc*128 + p
    wo_s = np.ascontiguousarray(
        wo.reshape(2, CQ, CQ).transpose(1, 0, 2)).astype(BF16)

    # bias^T tiles: bT[k, h, q] = bias[0,0,h,q,k]
    bT = bias[0, 0].transpose(2, 0, 1)            # [K, H, Q]
    eb2 = np.empty((CQ, 2, 4 * S), dtype=BF16)
    ebB = np.empty((CQ, 2, 4 * S), dtype=BF16)
    bA = np.empty((CQ, 2, 4 * S), dtype=np.float32)
    for kc in range(2):
        eb2[:, kc] = np.exp(
            bT[kc * CQ:(kc + 1) * CQ, 0:4, :]).reshape(CQ, 4 * S).astype(BF16)
        bA[:, kc] = (bT[kc * CQ:(kc + 1) * CQ, 4:8, :] * FE_A).reshape(CQ, 4 * S)
        ebB[:, kc] = np.exp(
            bT[kc * CQ:(kc + 1) * CQ, 4:8, :]).reshape(CQ, 4 * S).astype(BF16)

    bgt = np.ascontiguousarray(
        (bg.reshape(2, CQ) * 0.5).T).astype(np.float32)   # [128, 2]
    bo_f = bo.reshape(1, CQ).astype(np.float32)

    in_maps = []
    for ci in range(NCORES):
        sl = slice(ci * NPER, (ci + 1) * NPER)
        xq = input_q[0, sl].transpose(0, 2, 1)
        xk = input_k[0, sl].transpose(0, 2, 1)
        xv = input_v[0, sl].transpose(0, 2, 1)
        x_np = np.ascontiguousarray(
            np.stack([xq, xk, xv], axis=1)).astype(BF16)  # [NPER,3,128,256]
        m = mask[0, sl, 0, 0, :]                          # [NPER, 256]
        emf = np.exp((m - 1.0) * 1.0e9)
        emf = np.ascontiguousarray(
            emf.T.reshape(2, CQ, NPER).transpose(1, 0, 2).reshape(CQ, 2 * NPER)
        ).astype(np.float32)
        in_maps.append({
            "x_all": x_np, "wq": wq_s, "wk": wk_s, "wv": wv_s, "wg": wg_s,
            "wo": wo_s, "eb2": eb2, "bA": bA, "ebB": ebB, "bgt": bgt, "bo": bo_f,
            "emf": emf,
        })

    res = run_bass_kernel_spmd(nc, in_maps, list(range(NCORES)))
    out = np.concatenate([r["out"][None] for r in res.results], axis=0)
    return out.reshape(1, N, S, CQ).astype(np.float32)


if __name__ == "__main__":
    rng = np.random.default_rng(0)
    inps = {
        "input_q": rng.standard_normal((B, N, S, CQ), dtype=np.float32),
        "input_k": rng.standard_normal((B, N, S, CQ), dtype=np.float32),
        "input_v": rng.standard_normal((B, N, S, CQ), dtype=np.float32),
        "mask": np.ones((B, N, 1, 1, S), dtype=np.float32),
        "bias": rng.standard_normal((B, 1, H, S, S), dtype=np.float32),
        "wq": rng.standard_normal((CQ, H * C), dtype=np.float32) * 0.05,
        "wk": rng.standard_normal((CQ, H * C), dtype=np.float32) * 0.05,
        "wv": rng.standard_normal((CQ, H * C), dtype=np.float32) * 0.05,
        "wg": rng.standard_normal((CQ, H * C), dtype=np.float32) * 0.05,
        "bg": np.ones((H * C,), dtype=np.float32),
        "wo": rng.standard_normal((H * C, CQ), dtype=np.float32) * 0.05,
        "bo": np.zeros((CQ,), dtype=np.float32),
    }
    out = kernel(**inps)
    print("out shape", out.shape, out.dtype, float(np.abs(out).mean()))


# revision 11
# speedup vs baseline: 1.1691x; 1.1691x over previous
"""Trainium2 Bass kernel for gated pair-bias attention (AlphaFold-style).

Reference computation per (b=1, n) row:
  q,k,v = proj(input_*) reshaped to [H=8, S=256, C=32]; q /= sqrt(32)
  a = softmax(q@k^T + (mask-1)*1e9 + bias)      # [H, Q, K]
  o = (a@v) * sigmoid(input_q@wg + bg)          # gated
  out = o @ wo + bo                             # [S, 128]

Sharding: dim 1 (N=256 rows) split across 8 cores, 32 rows/core.

Engine-balanced design (v3). Baseline was Act(exp)+DVE(bias-mult) bound:
  - direct qT/kT (no block-diag q): per-head logits matmuls (c=32, m=256)
  - exp(l)*exp(b) product form for 3 of 4 logit tiles (Act engine),
    4th tile via the i16 fast-exp pun on DVE: a single AFFINE_THEN_ADD
    custom op computes i16 = l*(128/ln2) + (b*(128/ln2) + 16256 - 7.5),
    whose int16 bits reinterpreted as bf16 equal exp(l+b) to ~2%.
  - gate computed transposed (gT = tanh-sigmoid of wg^T x), fixup on DVE
  - AV natural with Z ride-along column (v_aug | 1), PSUM per q-chunk
  - normalize (1/Z) on Pool, PE transpose, gate-mult on DVE (2x bf16)
  - final matmul from og^T, bias-add + evac on Pool, DMA out
Work is spread so PE/Act/DVE/Pool all run ~equally busy.
"""

import math
import sys

sys.path.insert(0, "/opt/trn_rl_repo")

import numpy as np
import ml_dtypes

BF16 = ml_dtypes.bfloat16

B, N, S, CQ = 1, 256, 256, 128
H, C = 8, 32
NCORES = 8
NPER = N // NCORES  # 32 rows per core

FE_A = 128.0 / math.log(2.0)   # fast-exp scale
FE_B = 127.0 * 128.0 - 7.5     # fast-exp bias (incl. sawtooth centering)
USE_FAST_EXP = True


def _build_bass():
    import concourse.bass as bass
    import concourse.bacc as bacc
    import concourse.tile as tile
    from concourse import mybir
    from concourse.masks import make_identity
    from concourse.dve_ops import AFFINE_THEN_ADD

    dt = mybir.dt
    AF = mybir.ActivationFunctionType
    ALU = mybir.AluOpType

    nc = bacc.Bacc()

    x_all = nc.declare_dram_parameter("x_all", [NPER, 3, CQ, S], dt.bfloat16, isOutput=False)
    wq_d = nc.declare_dram_parameter("wq", [CQ, 2, CQ], dt.bfloat16, isOutput=False)
    wk_d = nc.declare_dram_parameter("wk", [CQ, 2, CQ], dt.bfloat16, isOutput=False)
    wv_d = nc.declare_dram_parameter("wv", [CQ, 2 * CQ], dt.bfloat16, isOutput=False)
    wg_d = nc.declare_dram_parameter("wg", [CQ, 2, CQ], dt.bfloat16, isOutput=False)
    wo_d = nc.declare_dram_parameter("wo", [CQ, 2, CQ], dt.bfloat16, isOutput=False)
    eb_d = nc.declare_dram_parameter("eb2", [CQ, 2, 4 * S], dt.bfloat16, isOutput=False)
    bA_d = nc.declare_dram_parameter("bA", [CQ, 2, 4 * S], dt.float32, isOutput=False)
    ebB_d = nc.declare_dram_parameter("ebB", [CQ, 2, 4 * S], dt.bfloat16, isOutput=False)
    bgt_d = nc.declare_dram_parameter("bgt", [CQ, 2], dt.float32, isOutput=False)
    bo_d = nc.declare_dram_parameter("bo", [1, CQ], dt.float32, isOutput=False)
    emf_d = nc.declare_dram_parameter("emf", [CQ, 2 * NPER], dt.float32, isOutput=False)
    out_d = nc.declare_dram_parameter("out", [NPER, S, CQ], dt.float32, isOutput=True)

    with tile.TileContext(nc) as tc:
        with (
            tc.tile_pool(name="const", bufs=1) as const,
            tc.tile_pool(name="xp", bufs=4) as xp,
            tc.tile_pool(name="qkp", bufs=3) as qkp,
            tc.tile_pool(name="vap", bufs=3) as vap,
            tc.tile_pool(name="gp", bufs=3) as gpool,
            tc.tile_pool(name="esb", bufs=8) as esb,
            tc.tile_pool(name="ogp", bufs=4) as ogp,
            tc.tile_pool(name="zp", bufs=3) as zp,
            tc.tile_pool(name="outp", bufs=3) as outp,
            tc.tile_pool(name="pse", bufs=2, space="PSUM") as pse,
            tc.tile_pool(name="pso", bufs=2, space="PSUM") as pso,
        ):
            # ---------- once-per-core setup ----------
            wq_t = const.tile([CQ, 2, CQ], dt.bfloat16, tag="wq")
            wk_t = const.tile([CQ, 2, CQ], dt.bfloat16, tag="wk")
            wv_t = const.tile([CQ, 2 * CQ], dt.bfloat16, tag="wv")
            wg_t = const.tile([CQ, 2, CQ], dt.bfloat16, tag="wg")
            wo_t = const.tile([CQ, 2, CQ], dt.bfloat16, tag="wo")
            eb_t = const.tile([CQ, 2, 4 * S], dt.bfloat16, tag="eb")
            bA_t = const.tile([CQ, 2, 4 * S], dt.float32, tag="bA")
            ebB_t = const.tile([CQ, 2, 4 * S], dt.bfloat16, tag="ebB")
            bgt_t = const.tile([CQ, 2], dt.float32, tag="bgt")
            bo_t = const.tile([CQ, CQ], dt.float32, tag="bo")
            emf = const.tile([CQ, 2, NPER], dt.float32, tag="emf")
            emb = const.tile([CQ, 2, NPER], dt.bfloat16, tag="emb")
            ident = const.tile([CQ, CQ], dt.bfloat16, tag="ident")

            nc.sync.dma_start(out=wq_t, in_=wq_d[:, :])
            nc.sync.dma_start(out=wk_t, in_=wk_d[:, :])
            nc.sync.dma_start(out=wv_t, in_=wv_d[:, :])
            nc.sync.dma_start(out=wg_t, in_=wg_d[:, :])
            nc.sync.dma_start(out=wo_t, in_=wo_d[:, :])
            nc.sync.dma_start(out=eb_t, in_=eb_d[:, :])
            nc.sync.dma_start(out=bA_t, in_=bA_d[:, :])
            nc.sync.dma_start(out=ebB_t, in_=ebB_d[:, :])
            nc.sync.dma_start(out=bgt_t, in_=bgt_d[:, :])
            nc.sync.dma_start(out=emf, in_=emf_d.rearrange("p (k n) -> p k n", k=2))
            bo_ap0 = bo_d[:, :]
            bo_bc_ap = bass.AP(tensor=bo_ap0.tensor, offset=bo_ap0.offset,
                               ap=[[0, CQ], [1, CQ]])
            nc.sync.dma_start(out=bo_t, in_=bo_bc_ap)
            make_identity(nc, ident)
            nc.vector.tensor_copy(out=emb, in_=emf)

            # ---------- per-row pipeline ----------
            for n in range(NPER):
                xt = xp.tile([CQ, 3, S], dt.bfloat16, tag="xt")
                nc.sync.dma_start(out=xt, in_=x_all[n].rearrange("t p s -> p t s"))

                # qT/kT [d, s] in one 2-bank PSUM tile: [:,0:2]=qT dc, [:,2:4]=kT dc
                pqk = pse.tile([CQ, 4, S], dt.float32, tag="proj", bufs=1)
                for dc in range(2):
                    nc.tensor.matmul(pqk[:, dc], wq_t[:, dc], xt[:, 0],
                                     start=True, stop=True)
                    nc.tensor.matmul(pqk[:, 2 + dc], wk_t[:, dc], xt[:, 1],
                                     start=True, stop=True)
                qkT = qkp.tile([CQ, 4, S], dt.bfloat16, tag="qkT")
                nc.scalar.activation(qkT.rearrange("p a s -> p (a s)"),
                                     pqk.rearrange("p a s -> p (a s)"), AF.Copy)
                # per-head shift: head-block g (partitions g*32..) -> partitions
                # 0-31 of qkT_s[:, g] so logits matmuls read base-0 operands
                qkT_s = qkp.tile([32, 4, 4, S], dt.bfloat16, tag="qkTs")
                for g in range(4):
                    eng = nc.sync if g % 2 == 0 else nc.gpsimd
                    eng.dma_start(out=qkT_s[:, g],
                                  in_=qkT[g * 32:(g + 1) * 32])

                # v natural [k, hd] per kc + gate transposed [hd, q] per dc
                pvg = pse.tile([CQ, 4, S], dt.float32, tag="proj", bufs=1)
                for kc in range(2):
                    nc.tensor.matmul(pvg[:, kc], xt[:, 2, kc * CQ:(kc + 1) * CQ],
                                     wv_t, start=True, stop=True)
                for dc in range(2):
                    nc.tensor.matmul(pvg[:, 2 + dc], wg_t[:, dc], xt[:, 0],
                                     start=True, stop=True)

                # v_aug [k, kc, h, 33]: v*em | em  (em==1 when mask is ones)
                va = vap.tile([CQ, 2, H, 33], dt.bfloat16, tag="va")
                for kc in range(2):
                    nc.scalar.activation(
                        va[:, kc, :, 0:32],
                        pvg[:, kc].rearrange("p (h x) -> p h x", x=32),
                        AF.Copy, scale=emf[:, kc, n:n + 1])
                nc.gpsimd.tensor_copy(
                    out=va[:, :, :, 32],
                    in_=emb[:, :, n:n + 1].broadcast_to((CQ, 2, H)))

                # gate: tanh((z+bg)/2) on Act; 0.5*t+0.5 fixup on DVE
                gt = gpool.tile([CQ, 2, S], dt.bfloat16, tag="gt")
                for dc in range(2):
                    nc.scalar.activation(gt[:, dc], pvg[:, 2 + dc], AF.Tanh,
                                         scale=0.5, bias=bgt_t[:, dc:dc + 1])
                g_sb = gpool.tile([CQ, 2, S], dt.bfloat16, tag="g")
                nc.gpsimd.tensor_scalar(out=g_sb, in0=gt, scalar1=0.5, scalar2=0.5,
                                        op0=ALU.mult, op1=ALU.add)

                # logits per (dc,kc): 4 heads, e^T[k, (h, q)]
                e_sb = {}
                for dc in range(2):
                    for kc in range(2):
                        pe_e = pse.tile([CQ, 4, S], dt.float32, tag="pse")
                        for hh in range(4):
                            nc.tensor.matmul(
                                pe_e[:, hh],
                                qkT_s[:, hh, 2 + dc, kc * CQ:(kc + 1) * CQ],
                                qkT_s[:, hh, dc, :],
                                start=True, stop=True)
                        et = esb.tile([CQ, 4, S], dt.bfloat16, tag="et")
                        flat_in = pe_e.rearrange("p h s -> p (h s)")
                        flat_out = et.rearrange("p h s -> p (h s)")
                        if dc == 1 and USE_FAST_EXP:
                            # fast-exp: i16 = l*A + (b*A + B); bits = bf16 e^(l+b)
                            nc.vector._custom_dve(
                                AFFINE_THEN_ADD,
                                out=flat_out.bitcast(dt.int16),
                                in0=flat_in, in1=bA_t[:, kc],
                                s0=FE_A, s1=FE_B)
                        elif dc == 1:
                            nc.scalar.activation(flat_out, flat_in, AF.Exp)
                            eng = nc.vector if kc == 0 else nc.gpsimd
                            eng.tensor_mul(flat_out, flat_out, ebB_t[:, kc])
                        else:
                            nc.scalar.activation(flat_out, flat_in, AF.Exp)
                            eng = nc.vector if kc == 0 else nc.gpsimd
                            eng.tensor_mul(
                                flat_out, flat_out, eb_t[:, kc])
                        e_sb[(dc, kc)] = et

                # AV with Z column: o[q, (h,33)] per q-chunk
                pos = []
                for qc in range(2):
                    po = pso.tile([CQ, H, 33], dt.float32, tag="pso")
                    for h in range(H):
                        dc, hh = h // 4, h % 4
                        for kc in range(2):
                            nc.tensor.matmul(
                                po[:, h],
                                e_sb[(dc, kc)][:, hh, qc * CQ:(qc + 1) * CQ],
                                va[:, kc, h],
                                start=(kc == 0), stop=(kc == 1))
                    pos.append(po)

                # normalize: rz = 1/Z (DVE), og1 = o * rz (Pool)
                rz = zp.tile([CQ, 2, H], dt.float32, tag="rz")
                og1 = ogp.tile([CQ, 2, H, 32], dt.bfloat16, tag="og1")
                for qc in range(2):
                    nc.vector.reciprocal(out=rz[:, qc], in_=pos[qc][:, :, 32])
                    nc.vector.tensor_mul(
                        og1[:, qc], pos[qc][:, :, 0:32],
                        rz[:, qc].unsqueeze(2).broadcast_to((CQ, H, 32)))

                # og^T via PE transpose, gate-mult on DVE (bf16 2x)
                ptp_raw = pso.tile([CQ, H, 33], dt.float32, tag="pso")
                ptp = ptp_raw.rearrange("p h x -> p (h x)").bitcast(
                    dt.bfloat16)[:, 0:512].rearrange(
                    "p (a b c) -> p a b c", a=2, b=2)
                og1f = og1.rearrange("p a h x -> p a (h x)")
                for dc in range(2):
                    for qc in range(2):
                        nc.tensor.transpose(
                            ptp[:, dc, qc],
                            og1f[:, qc, dc * CQ:(dc + 1) * CQ], ident)
                ogT = ogp.tile([CQ, 2, 2, CQ], dt.bfloat16, tag="ogT")
                for dc in range(2):
                    nc.vector.tensor_mul(ogT[:, dc], ptp[:, dc],
                                         g_sb[:, dc].rearrange("p (a c) -> p a c", c=CQ))

                # final: out[q, c] = og^T.T @ wo + bo
                pout_raw = pso.tile([CQ, H, 33], dt.float32, tag="pso")
                pout = pout_raw.rearrange("p h x -> p (h x)")[:, 0:256].rearrange(
                    "p (a c) -> p a c", a=2)
                for qc in range(2):
                    for dc in range(2):
                        nc.tensor.matmul(pout[:, qc], ogT[:, dc, qc], wo_t[:, dc],
                                         start=(dc == 0), stop=(dc == 1))
                out_sb = outp.tile([CQ, 2, CQ], dt.float32, tag="osb")
                nc.vector.tensor_add(
                    out_sb, pout, bo_t.unsqueeze(1).broadcast_to((CQ, 2, CQ)))
                nc.sync.dma_start(
                    out=out_d[n].rearrange("(qc p) c -> p qc c", p=CQ),
                    in_=out_sb)
    if not nc.is_finalized():
        nc.finalize()
    return nc


_NC_CACHE = None


def _get_nc():
    global _NC_CACHE
    if _NC_CACHE is None:
        _NC_CACHE = _build_bass()
    return _NC_CACHE


def kernel(input_q, input_k, input_v, mask, bias, wq, wk, wv, wg, bg, wo, bo):
    from concourse.bass_utils import run_bass_kernel_spmd

    nc = _get_nc()

    # ---- host-side prep (sharding + layout) ----
    wq_s = np.ascontiguousarray(
        (wq / math.sqrt(C)).reshape(CQ, 2, CQ)).astype(BF16)
    wk_s = np.ascontiguousarray(wk.reshape(CQ, 2, CQ)).astype(BF16)
    wv_s = wv.astype(BF16)
    wg_s = np.ascontiguousarray(wg.reshape(CQ, 2, CQ)).astype(BF16)
    # wo [hd, c] -> [p, dc, c] with hd = dc*128 + p
    wo_s = np.ascontiguousarray(
        wo.reshape(2, CQ, CQ).transpose(1, 0, 2)).astype(BF16)

    # bias^T tiles: bT[k, h, q] = bias[0,0,h,q,k]
    bT = bias[0, 0].transpose(2, 0, 1)            # [K, H, Q]
    eb2 = np.empty((CQ, 2, 4 * S), dtype=BF16)
    ebB = np.empty((CQ, 2, 4 * S), dtype=BF16)
    bA = np.empty((CQ, 2, 4 * S), dtype=np.float32)
    for kc in range(2):
        eb2[:, kc] = np.exp(
            bT[kc * CQ:(kc + 1) * CQ, 0:4, :]).reshape(CQ, 4 * S).astype(BF16)
        bA[:, kc] = (bT[kc * CQ:(kc + 1) * CQ, 4:8, :] * FE_A).reshape(CQ, 4 * S)
        ebB[:, kc] = np.exp(
            bT[kc * CQ:(kc + 1) * CQ, 4:8, :]).reshape(CQ, 4 * S).astype(BF16)

    bgt = np.ascontiguousarray(
        (bg.reshape(2, CQ) * 0.5).T).astype(np.float32)   # [128, 2]
    bo_f = bo.reshape(1, CQ).astype(np.float32)

    in_maps = []
    for ci in range(NCORES):
        sl = slice(ci * NPER, (ci + 1) * NPER)
        xq = input_q[0, sl].transpose(0, 2, 1)
        xk = input_k[0, sl].transpose(0, 2, 1)
        xv = input_v[0, sl].transpose(0, 2, 1)
        x_np = np.ascontiguousarray(
            np.stack([xq, xk, xv], axis=1)).astype(BF16)  # [NPER,3,128,256]
        m = mask[0, sl, 0, 0, :]                          # [NPER, 256]
        emf = np.exp((m - 1.0) * 1.0e9)
        emf = np.ascontiguousarray(
            emf.T.reshape(2, CQ, NPER).transpose(1, 0, 2).reshape(CQ, 2 * NPER)
        ).astype(np.float32)
        in_maps.append({
            "x_all": x_np, "wq": wq_s, "wk": wk_s, "wv": wv_s, "wg": wg_s,
            "wo": wo_s, "eb2": eb2, "bA": bA, "ebB": ebB, "bgt": bgt, "bo": bo_f,
            "emf": emf,
        })

    res = run_bass_kernel_spmd(nc, in_maps, list(range(NCORES)))
    out = np.concatenate([r["out"][None] for r in res.results], axis=0)
    return out.reshape(1, N, S, CQ).astype(np.float32)


if __name__ == "__main__":
    rng = np.random.default_rng(0)
    inps = {
        "input_q": rng.standard_normal((B, N, S, CQ), dtype=np.float32),
        "input_k": rng.standard_normal((B, N, S, CQ), dtype=np.float32),
        "input_v": rng.standard_normal((B, N, S, CQ), dtype=np.float32),
        "mask": np.ones((B, N, 1, 1, S), dtype=np.float32),
        "bias": rng.standard_normal((B, 1, H, S, S), dtype=np.float32),
        "wq": rng.standard_normal((CQ, H * C), dtype=np.float32) * 0.05,
        "wk": rng.standard_normal((CQ, H * C), dtype=np.float32) * 0.05,
        "wv": rng.standard_normal((CQ, H * C), dtype=np.float32) * 0.05,
        "wg": rng.standard_normal((CQ, H * C), dtype=np.float32) * 0.05,
        "bg": np.ones((H * C,), dtype=np.float32),
        "wo": rng.standard_normal((H * C, CQ), dtype=np.float32) * 0.05,
        "bo": np.zeros((CQ,), dtype=np.float32),
    }
    out = kernel(**inps)
    print("out shape", out.shape, out.dtype, float(np.abs(out).mean()))


# revision 12
# speedup vs baseline: 1.2865x; 1.1004x over previous
"""Trainium2 Bass kernel for gated pair-bias attention (AlphaFold-style).

Reference computation per (b=1, n) row:
  q,k,v = proj(input_*) reshaped to [H=8, S=256, C=32]; q /= sqrt(32)
  a = softmax(q@k^T + (mask-1)*1e9 + bias)      # [H, Q, K]
  o = (a@v) * sigmoid(input_q@wg + bg)          # gated
  out = o @ wo + bo                             # [S, 128]

Sharding: dim 1 (N=256 rows) split across 8 cores, 32 rows/core.

Engine-balanced design (v3). Baseline was Act(exp)+DVE(bias-mult) bound:
  - direct qT/kT (no block-diag q): per-head logits matmuls (c=32, m=256)
  - exp(l)*exp(b) product form for 3 of 4 logit tiles (Act engine),
    4th tile via the i16 fast-exp pun on DVE: a single AFFINE_THEN_ADD
    custom op computes i16 = l*(128/ln2) + (b*(128/ln2) + 16256 - 7.5),
    whose int16 bits reinterpreted as bf16 equal exp(l+b) to ~2%.
  - gate computed transposed (gT = tanh-sigmoid of wg^T x), fixup on DVE
  - AV natural with Z ride-along column (v_aug | 1), PSUM per q-chunk
  - normalize (1/Z) on Pool, PE transpose, gate-mult on DVE (2x bf16)
  - final matmul from og^T, bias-add + evac on Pool, DMA out
Work is spread so PE/Act/DVE/Pool all run ~equally busy.
"""

import math
import sys

sys.path.insert(0, "/opt/trn_rl_repo")

import numpy as np
import ml_dtypes

BF16 = ml_dtypes.bfloat16

B, N, S, CQ = 1, 256, 256, 128
H, C = 8, 32
NCORES = 8
NPER = N // NCORES  # 32 rows per core

FE_A = 128.0 / math.log(2.0)   # fast-exp scale
FE_B = 127.0 * 128.0 - 7.5     # fast-exp bias (incl. sawtooth centering)
USE_FAST_EXP = True


def _build_bass():
    import concourse.bass as bass
    import concourse.bacc as bacc
    import concourse.tile as tile
    from concourse import mybir
    from concourse.masks import make_identity
    from concourse.dve_ops import AFFINE_THEN_ADD

    dt = mybir.dt
    AF = mybir.ActivationFunctionType
    ALU = mybir.AluOpType

    nc = bacc.Bacc()

    x_all = nc.declare_dram_parameter("x_all", [NPER, 3, CQ, S], dt.bfloat16, isOutput=False)
    wq_d = nc.declare_dram_parameter("wq", [CQ, 2, CQ], dt.bfloat16, isOutput=False)
    wk_d = nc.declare_dram_parameter("wk", [CQ, 2, CQ], dt.bfloat16, isOutput=False)
    wv_d = nc.declare_dram_parameter("wv", [CQ, 2 * CQ], dt.bfloat16, isOutput=False)
    wg_d = nc.declare_dram_parameter("wg", [CQ, 2, CQ], dt.bfloat16, isOutput=False)
    wo_d = nc.declare_dram_parameter("wo", [CQ, 2, CQ], dt.bfloat16, isOutput=False)
    eb_d = nc.declare_dram_parameter("eb2", [CQ, 2, 4 * S], dt.bfloat16, isOutput=False)
    bA_d = nc.declare_dram_parameter("bA", [CQ, 2, 4 * S], dt.float32, isOutput=False)
    ebB_d = nc.declare_dram_parameter("ebB", [CQ, 2, 4 * S], dt.bfloat16, isOutput=False)
    bgt_d = nc.declare_dram_parameter("bgt", [CQ, 2], dt.float32, isOutput=False)
    bo_d = nc.declare_dram_parameter("bo", [1, CQ], dt.float32, isOutput=False)
    emf_d = nc.declare_dram_parameter("emf", [CQ, 2 * NPER], dt.float32, isOutput=False)
    out_d = nc.declare_dram_parameter("out", [NPER, S, CQ], dt.float32, isOutput=True)

    with tile.TileContext(nc) as tc:
        with (
            tc.tile_pool(name="const", bufs=1) as const,
            tc.tile_pool(name="xp", bufs=4) as xp,
            tc.tile_pool(name="qkp", bufs=3) as qkp,
            tc.tile_pool(name="vap", bufs=3) as vap,
            tc.tile_pool(name="gp", bufs=3) as gpool,
            tc.tile_pool(name="esb", bufs=10) as esb,
            tc.tile_pool(name="ogp", bufs=4) as ogp,
            tc.tile_pool(name="zp", bufs=3) as zp,
            tc.tile_pool(name="outp", bufs=3) as outp,
            tc.tile_pool(name="pse", bufs=2, space="PSUM") as pse,
            tc.tile_pool(name="pso", bufs=2, space="PSUM") as pso,
        ):
            # ---------- once-per-core setup ----------
            wq_t = const.tile([CQ, 2, CQ], dt.bfloat16, tag="wq")
            wk_t = const.tile([CQ, 2, CQ], dt.bfloat16, tag="wk")
            wv_t = const.tile([CQ, 2 * CQ], dt.bfloat16, tag="wv")
            wg_t = const.tile([CQ, 2, CQ], dt.bfloat16, tag="wg")
            wo_t = const.tile([CQ, 2, CQ], dt.bfloat16, tag="wo")
            eb_t = const.tile([CQ, 2, 4 * S], dt.bfloat16, tag="eb")
            bA_t = const.tile([CQ, 2, 4 * S], dt.float32, tag="bA")
            ebB_t = const.tile([CQ, 2, 4 * S], dt.bfloat16, tag="ebB")
            bgt_t = const.tile([CQ, 2], dt.float32, tag="bgt")
            bo_t = const.tile([CQ, CQ], dt.float32, tag="bo")
            emf = const.tile([CQ, 2, NPER], dt.float32, tag="emf")
            emb = const.tile([CQ, 2, NPER], dt.bfloat16, tag="emb")
            ident = const.tile([CQ, CQ], dt.bfloat16, tag="ident")

            nc.sync.dma_start(out=wq_t, in_=wq_d[:, :])
            nc.sync.dma_start(out=wk_t, in_=wk_d[:, :])
            nc.sync.dma_start(out=wv_t, in_=wv_d[:, :])
            nc.sync.dma_start(out=wg_t, in_=wg_d[:, :])
            nc.sync.dma_start(out=wo_t, in_=wo_d[:, :])
            nc.sync.dma_start(out=eb_t, in_=eb_d[:, :])
            nc.sync.dma_start(out=bA_t, in_=bA_d[:, :])
            nc.sync.dma_start(out=ebB_t, in_=ebB_d[:, :])
            nc.sync.dma_start(out=bgt_t, in_=bgt_d[:, :])
            nc.sync.dma_start(out=emf, in_=emf_d.rearrange("p (k n) -> p k n", k=2))
            bo_ap0 = bo_d[:, :]
            bo_bc_ap = bass.AP(tensor=bo_ap0.tensor, offset=bo_ap0.offset,
                               ap=[[0, CQ], [1, CQ]])
            nc.sync.dma_start(out=bo_t, in_=bo_bc_ap)
            make_identity(nc, ident)
            nc.vector.tensor_copy(out=emb, in_=emf)

            # ---------- per-row pipeline ----------
            for n in range(NPER):
                xt = xp.tile([CQ, 3, S], dt.bfloat16, tag="xt")
                nc.sync.dma_start(out=xt, in_=x_all[n].rearrange("t p s -> p t s"))

                # qT/kT [d, s] in one 2-bank PSUM tile: [:,0:2]=qT dc, [:,2:4]=kT dc
                pqk = pse.tile([CQ, 4, S], dt.float32, tag="proj", bufs=1)
                for dc in range(2):
                    nc.tensor.matmul(pqk[:, dc], wq_t[:, dc], xt[:, 0],
                                     start=True, stop=True)
                    nc.tensor.matmul(pqk[:, 2 + dc], wk_t[:, dc], xt[:, 1],
                                     start=True, stop=True)
                qkT = qkp.tile([CQ, 4, S], dt.bfloat16, tag="qkT")
                for half in range(2):
                    nc.scalar.activation(
                        qkT[:, 2 * half:2 * half + 2].rearrange("p a s -> p (a s)"),
                        pqk[:, 2 * half:2 * half + 2].rearrange("p a s -> p (a s)"),
                        AF.Copy)
                # per-head shift: head-block g (partitions g*32..) -> partitions
                # 0-31 of qkT_s[:, g] so logits matmuls read base-0 operands
                qkT_s = qkp.tile([32, 4, 4, S], dt.bfloat16, tag="qkTs")
                for g in range(4):
                    nc.gpsimd.dma_start(out=qkT_s[:, g],
                                        in_=qkT[g * 32:(g + 1) * 32])

                # v natural [k, hd] per kc + gate transposed [hd, q] per dc
                pvg = pse.tile([CQ, 4, S], dt.float32, tag="proj", bufs=1)
                for kc in range(2):
                    nc.tensor.matmul(pvg[:, kc], xt[:, 2, kc * CQ:(kc + 1) * CQ],
                                     wv_t, start=True, stop=True)
                for dc in range(2):
                    nc.tensor.matmul(pvg[:, 2 + dc], wg_t[:, dc], xt[:, 0],
                                     start=True, stop=True)

                # v_aug [k, kc, h, 33]: v*em | em  (em==1 when mask is ones)
                va = vap.tile([CQ, 2, H, 33], dt.bfloat16, tag="va")
                for kc in range(2):
                    nc.scalar.activation(
                        va[:, kc, :, 0:32],
                        pvg[:, kc].rearrange("p (h x) -> p h x", x=32),
                        AF.Copy, scale=emf[:, kc, n:n + 1])
                nc.gpsimd.tensor_copy(
                    out=va[:, :, :, 32],
                    in_=emb[:, :, n:n + 1].broadcast_to((CQ, 2, H)))

                # gate: tanh((z+bg)/2) on Act; 0.5*t+0.5 fixup on DVE
                gt = gpool.tile([CQ, 2, S], dt.bfloat16, tag="gt")
                for dc in range(2):
                    nc.scalar.activation(gt[:, dc], pvg[:, 2 + dc], AF.Tanh,
                                         scale=0.5, bias=bgt_t[:, dc:dc + 1])
                g_sb = gpool.tile([CQ, 2, S], dt.bfloat16, tag="g")
                nc.gpsimd.tensor_scalar(out=g_sb, in0=gt, scalar1=0.5, scalar2=0.5,
                                        op0=ALU.mult, op1=ALU.add)

                # logits per (dc,kc): 4 heads, e^T[k, (h, q)]
                e_sb = {}
                for dc in range(2):
                    for kc in range(2):
                        pe_e = pse.tile([CQ, 4, S], dt.float32, tag="pse")
                        for hh in range(4):
                            nc.tensor.matmul(
                                pe_e[:, hh],
                                qkT_s[:, hh, 2 + dc, kc * CQ:(kc + 1) * CQ],
                                qkT_s[:, hh, dc, :],
                                start=True, stop=True)
                        et = esb.tile([CQ, 4, S], dt.bfloat16, tag="et")
                        flat_in = pe_e.rearrange("p h s -> p (h s)")
                        flat_out = et.rearrange("p h s -> p (h s)")
                        if dc == 1 and USE_FAST_EXP:
                            # fast-exp: i16 = l*A + (b*A + B); bits = bf16 e^(l+b)
                            nc.vector._custom_dve(
                                AFFINE_THEN_ADD,
                                out=flat_out.bitcast(dt.int16),
                                in0=flat_in, in1=bA_t[:, kc],
                                s0=FE_A, s1=FE_B)
                        elif dc == 1:
                            nc.scalar.activation(flat_out, flat_in, AF.Exp)
                            eng = nc.vector if kc == 0 else nc.gpsimd
                            eng.tensor_mul(flat_out, flat_out, ebB_t[:, kc])
                        else:
                            nc.scalar.activation(flat_out, flat_in, AF.Exp)
                            nc.vector.tensor_mul(
                                flat_out, flat_out, eb_t[:, kc])
                        e_sb[(dc, kc)] = et

                # AV with Z column: o[q, (h,33)] per q-chunk
                pos = []
                for qc in range(2):
                    po = pso.tile([CQ, H, 33], dt.float32, tag="pso")
                    for h in range(H):
                        dc, hh = h // 4, h % 4
                        for kc in range(2):
                            nc.tensor.matmul(
                                po[:, h],
                                e_sb[(dc, kc)][:, hh, qc * CQ:(qc + 1) * CQ],
                                va[:, kc, h],
                                start=(kc == 0), stop=(kc == 1))
                    pos.append(po)

                # normalize: rz = 1/Z (DVE), og1 = o * rz (Pool)
                rz = zp.tile([CQ, 2, H], dt.float32, tag="rz")
                og1 = ogp.tile([CQ, 2, H, 32], dt.bfloat16, tag="og1")
                for qc in range(2):
                    nc.vector.reciprocal(out=rz[:, qc], in_=pos[qc][:, :, 32])
                    nc.vector.tensor_mul(
                        og1[:, qc], pos[qc][:, :, 0:32],
                        rz[:, qc].unsqueeze(2).broadcast_to((CQ, H, 32)))

                # og^T via PE transpose, gate-mult on DVE (bf16 2x)
                ptp_raw = pso.tile([CQ, H, 33], dt.float32, tag="pso")
                ptp = ptp_raw.rearrange("p h x -> p (h x)").bitcast(
                    dt.bfloat16)[:, 0:512].rearrange(
                    "p (a b c) -> p a b c", a=2, b=2)
                og1f = og1.rearrange("p a h x -> p a (h x)")
                for dc in range(2):
                    for qc in range(2):
                        nc.tensor.transpose(
                            ptp[:, dc, qc],
                            og1f[:, qc, dc * CQ:(dc + 1) * CQ], ident)
                ogT = ogp.tile([CQ, 2, 2, CQ], dt.bfloat16, tag="ogT")
                for dc in range(2):
                    nc.vector.tensor_mul(ogT[:, dc], ptp[:, dc],
                                         g_sb[:, dc].rearrange("p (a c) -> p a c", c=CQ))

                # final: out[q, c] = og^T.T @ wo + bo
                pout_raw = pso.tile([CQ, H, 33], dt.float32, tag="pso")
                pout = pout_raw.rearrange("p h x -> p (h x)")[:, 0:256].rearrange(
                    "p (a c) -> p a c", a=2)
                for qc in range(2):
                    for dc in range(2):
                        nc.tensor.matmul(pout[:, qc], ogT[:, dc, qc], wo_t[:, dc],
                                         start=(dc == 0), stop=(dc == 1))
                out_sb = outp.tile([CQ, 2, CQ], dt.float32, tag="osb")
                nc.vector.tensor_add(
                    out_sb, pout, bo_t.unsqueeze(1).broadcast_to((CQ, 2, CQ)))
                nc.sync.dma_start(
                    out=out_d[n].rearrange("(qc p) c -> p qc c", p=CQ),
                    in_=out_sb)
    if not nc.is_finalized():
        nc.finalize()
    return nc


_NC_CACHE = None


def _get_nc():
    global _NC_CACHE
    if _NC_CACHE is None:
        _NC_CACHE = _build_bass()
    return _NC_CACHE


def kernel(input_q, input_k, input_v, mask, bias, wq, wk, wv, wg, bg, wo, bo):
    from concourse.bass_utils import run_bass_kernel_spmd

    nc = _get_nc()

    # ---- host-side prep (sharding + layout) ----
    wq_s = np.ascontiguousarray(
        (wq / math.sqrt(C)).reshape(CQ, 2, CQ)).astype(BF16)
    wk_s = np.ascontiguousarray(wk.reshape(CQ, 2, CQ)).astype(BF16)
    wv_s = wv.astype(BF16)
    wg_s = np.ascontiguousarray(wg.reshape(CQ, 2, CQ)).astype(BF16)
    # wo [hd, c] -> [p, dc, c] with hd = dc*128 + p
    wo_s = np.ascontiguousarray(
        wo.reshape(2, CQ, CQ).transpose(1, 0, 2)).astype(BF16)

    # bias^T tiles: bT[k, h, q] = bias[0,0,h,q,k]
    bT = bias[0, 0].transpose(2, 0, 1)            # [K, H, Q]
    eb2 = np.empty((CQ, 2, 4 * S), dtype=BF16)
    ebB = np.empty((CQ, 2, 4 * S), dtype=BF16)
    bA = np.empty((CQ, 2, 4 * S), dtype=np.float32)
    for kc in range(2):
        eb2[:, kc] = np.exp(
            bT[kc * CQ:(kc + 1) * CQ, 0:4, :]).reshape(CQ, 4 * S).astype(BF16)
        bA[:, kc] = (bT[kc * CQ:(kc + 1) * CQ, 4:8, :] * FE_A).reshape(CQ, 4 * S)
        ebB[:, kc] = np.exp(
            bT[kc * CQ:(kc + 1) * CQ, 4:8, :]).reshape(CQ, 4 * S).astype(BF16)

    bgt = np.ascontiguousarray(
        (bg.reshape(2, CQ) * 0.5).T).astype(np.float32)   # [128, 2]
    bo_f = bo.reshape(1, CQ).astype(np.float32)

    in_maps = []
    for ci in range(NCORES):
        sl = slice(ci * NPER, (ci + 1) * NPER)
        xq = input_q[0, sl].transpose(0, 2, 1)
        xk = input_k[0, sl].transpose(0, 2, 1)
        xv = input_v[0, sl].transpose(0, 2, 1)
        x_np = np.ascontiguousarray(
            np.stack([xq, xk, xv], axis=1)).astype(BF16)  # [NPER,3,128,256]
        m = mask[0, sl, 0, 0, :]                          # [NPER, 256]
        emf = np.exp((m - 1.0) * 1.0e9)
        emf = np.ascontiguousarray(
            emf.T.reshape(2, CQ, NPER).transpose(1, 0, 2).reshape(CQ, 2 * NPER)
        ).astype(np.float32)
        in_maps.append({
            "x_all": x_np, "wq": wq_s, "wk": wk_s, "wv": wv_s, "wg": wg_s,
            "wo": wo_s, "eb2": eb2, "bA": bA, "ebB": ebB, "bgt": bgt, "bo": bo_f,
            "emf": emf,
        })

    res = run_bass_kernel_spmd(nc, in_maps, list(range(NCORES)))
    out = np.concatenate([r["out"][None] for r in res.results], axis=0)
    return out.reshape(1, N, S, CQ).astype(np.float32)


if __name__ == "__main__":
    rng = np.random.default_rng(0)
    inps = {
        "input_q": rng.standard_normal((B, N, S, CQ), dtype=np.float32),
        "input_k": rng.standard_normal((B, N, S, CQ), dtype=np.float32),
        "input_v": rng.standard_normal((B, N, S, CQ), dtype=np.float32),
        "mask": np.ones((B, N, 1, 1, S), dtype=np.float32),
        "bias": rng.standard_normal((B, 1, H, S, S), dtype=np.float32),
        "wq": rng.standard_normal((CQ, H * C), dtype=np.float32) * 0.05,
        "wk": rng.standard_normal((CQ, H * C), dtype=np.float32) * 0.05,
        "wv": rng.standard_normal((CQ, H * C), dtype=np.float32) * 0.05,
        "wg": rng.standard_normal((CQ, H * C), dtype=np.float32) * 0.05,
        "bg": np.ones((H * C,), dtype=np.float32),
        "wo": rng.standard_normal((H * C, CQ), dtype=np.float32) * 0.05,
        "bo": np.zeros((CQ,), dtype=np.float32),
    }
    out = kernel(**inps)
    print("out shape", out.shape, out.dtype, float(np.abs(out).mean()))
